# revision 85
# baseline (speedup 1.0000x reference)
"""Trainium2 Bass kernel for nn_AttentionBlock (sparse_attention).

Reference computation (N=8192, D=256):
    q = l2norm(x @ Wq.T + bq); k = l2norm(x @ Wk.T + bk); v = x @ Wv.T + bv
    w = relu(q @ k.T); w[diag] = 0; w /= max(rowsum(w), eps)
    out = w @ v + x

Algebraic restructuring (same as the bf16 baseline):
  * relu is positively homogeneous and rows are renormalized by their sum,
    so the q-normalization scale cancels: skip it entirely.
  * The k-normalization column scale cs_j = 1/|k_j| commutes through relu.
    It is folded into v (v rows scaled by cs_j at the v evacuation) and
    carried as an fp8 copy of cs for the denominator row sums
    (flash-attention ones-trick, with cs8 as the moving operand).
  * The zeroed diagonal is handled by subtracting a separately computed self
    term m_r = relu(q_r . k_r)/|k_r| from numerator (m_r * v_r) and row sum.
  * v bias folded into the host-side residual (xr + bv).

Speed: all big matmuls run in fp8 (e4m3) with MatmulPerfMode.DoubleRow:
one matmul contracts 2x128 partitions at 0.5 cycles/row, 4x fewer PE
cycles than bf16 pairing for the same D=256 contraction.  fp8 operands
live in "pair layout" [128, 2, n]: partition p, pair i holds contraction
element i*128+p.  Scores for 4 j-blocks accumulate in one 2-bank PSUM
tile [128, 1024] and leave through a single pure-relu evacuation into
wt4 [128, 4, 256] fp8, which directly exposes the DoubleRow stationary
pairs for the w @ v matmul.

The PSUM evacuations (relu on scores, bias-adds on k/q, cs-scaled copies
of v) are the bottleneck; only DVE and ACT can read PSUM, so they split
that work while GPSIMD (Pool) takes SBUF-side work (k^2 for the column
norms, q*k self-term products, epilogue arithmetic).  The kernel runs in
two sequential phases so each gets the PSUM banks it needs: phase A
(projections; 6 rotating work banks) then phase B (scores + w@v; 3
double-bank score tiles + 2 accumulator banks).

Engine assignment is round-robin per stream (SC/KV/VS/KSQ/QS patterns
below), tuned against the Tile scheduler's cost-model makespan; the
strict DVE/ACT alternation schedules best.

Measured on the 8-core SPMD run: rel err 3.07e-3 vs the fp32 reference
(gate 2e-2); cost-model makespan 76326 ns/core (bf16 baseline: 159647).
"""

import numpy as np

import concourse.bass as bass
import concourse.bacc as bacc
import concourse.mybir as mybir
from concourse import tile
from concourse.bass_utils import run_bass_kernel_spmd

F32 = mybir.dt.float32
BF16 = mybir.dt.bfloat16
F8 = mybir.dt.float8e4
AF = mybir.ActivationFunctionType
PM = mybir.MatmulPerfMode
ALU = mybir.AluOpType

M = 8       # cores
N = 8192    # tokens
D = 256     # feature dim

TRACE = False
LAST = None
_CACHE = {}

# engine split knobs (tuned against the scheduler makespan)
import os
SC_PAT = os.environ.get("K_SC", "DA")
KV_PAT = os.environ.get("K_KV", "DA")    # k/q/ksf/vself/v psum evacs
VS_PAT = os.environ.get("K_VS", "P")     # in-place cs scaling of v (SBUF)
KSQ_PAT = os.environ.get("K_KSQ", "PDPP")  # ksq: P=Pool, D=DVE
QS_PAT = os.environ.get("K_QS", "A")     # qs squares: A=ACT, P=Pool


def build(n=N, r=N // M):
    NJ = n // 128            # 64  j blocks
    CH = n // 1024           # 8   xT streaming chunks (1024 j each)
    RT = r // 128            # 8   128-row tiles of this core's rows
    RW = 256                 # row-block width (r cols per score group)
    NRB = r // RW            # 4   row blocks
    SS = RW // 128           # 2   128-row subtiles per row block
    GJB = 4                  # j blocks per score psum group
    NG = NJ // GJB           # 16  score groups per row block

    nc = bacc.Bacc(None)
    xTp_d = nc.declare_dram_parameter("xTp", [128, 2, n], F8, isOutput=False)
    xrTp_d = nc.declare_dram_parameter("xrTp", [128, 2, r], F8, isOutput=False)
    xr_d = nc.declare_dram_parameter("xr", [r, D], F32, isOutput=False)
    wq_d = nc.declare_dram_parameter("wqTp", [128, 2, D], F8, isOutput=False)
    wk_d = nc.declare_dram_parameter("wkTp", [128, 2, D], F8, isOutput=False)
    wv_d = nc.declare_dram_parameter("wvTp", [128, 2, D], F8, isOutput=False)
    bq_d = nc.declare_dram_parameter("bq2", [128, 2], F32, isOutput=False)
    bk_d = nc.declare_dram_parameter("bk2", [128, 2], F32, isOutput=False)
    out_d = nc.declare_dram_parameter("out", [r, D], F32, isOutput=True)

    def mk_cycle(pat, m):
        state = {"i": 0}
        def nxt():
            e = m[pat[state["i"] % len(pat)]]
            state["i"] += 1
            return e
        return nxt

    with tile.TileContext(nc, pool_alloc_mode="queue") as tc:
        B = lambda k, d: int(os.environ.get(k, d))
        with tc.tile_pool(name="pers", bufs=1) as pers, \
             tc.tile_pool(name="xtp", bufs=B("K_XT", 2)) as xtp, \
             tc.tile_pool(name="ksqp", bufs=B("K_KSQB", 2)) as ksqp, \
             tc.tile_pool(name="wtp", bufs=B("K_WT", 4)) as wtp, \
             tc.tile_pool(name="ep", bufs=B("K_EP", 2)) as ep, \
             tc.tile_pool(name="otp", bufs=2) as otp:
            emap = {"A": nc.scalar, "D": nc.vector, "P": nc.gpsimd, "S": "S"}
            sc_eng = mk_cycle(SC_PAT, emap)
            kv_eng = mk_cycle(KV_PAT, emap)
            vs_eng = mk_cycle(VS_PAT, emap)
            ksq_eng = mk_cycle(KSQ_PAT, emap)

            # ---- persistent SBUF state ----
            kTp = pers.tile([128, 2, n], F8, name="kTp", tag="kTp")
            qTp = pers.tile([128, 2, r], F8, name="qTp", tag="qTp")
            vp = [pers.tile([128, 2, D], F8, name=f"vp{jj}", tag=f"vp{jj}")
                  for jj in range(NJ // 2)]
            cs = pers.tile([128, NJ], F32, name="cs", tag="cs")
            cs8 = pers.tile([128, NJ, 1], F8, name="cs8", tag="cs8")
            wqt = pers.tile([128, 2, D], F8, name="wqt", tag="wqt")
            wkt = pers.tile([128, 2, D], F8, name="wkt", tag="wkt")
            wvt = pers.tile([128, 2, D], F8, name="wvt", tag="wvt")
            bq2 = pers.tile([128, 2], F32, name="bq2", tag="bq2")
            bk2 = pers.tile([128, 2], F32, name="bk2", tag="bk2")
            ones8 = pers.tile([128, 2, 1], F8, name="ones8", tag="ones8")
            ksf = pers.tile([128, 2, r], BF16, name="ksf", tag="ksf")
            qk8 = pers.tile([128, 2, r], F8, name="qk8", tag="qk8")
            qs8 = pers.tile([128, 2, r], F8, name="qs8", tag="qs8")
            vself = [pers.tile([128, 2, D], F32, name=f"vs{t}", tag=f"vs{t}")
                     for t in range(RT // 2)]
            xrt = [pers.tile([128, D], F32, name=f"xrs{t}", tag=f"xrs{t}")
                   for t in range(RT)]
            msb = pers.tile([128, RT], F32, name="msb", tag="msb")
            xrTp = pers.tile([128, 2, r], F8, name="xrTp", tag="xrTp")

            nc.gpsimd.dma_start(wkt[:], wk_d[:])
            nc.gpsimd.dma_start(wvt[:], wv_d[:])
            nc.gpsimd.dma_start(wqt[:], wq_d[:])
            nc.gpsimd.dma_start(xrTp[:], xrTp_d[:])
            nc.gpsimd.dma_start(bk2[:], bk_d[:])
            nc.gpsimd.dma_start(bq2[:], bq_d[:])
            nc.vector.memset(ones8[:], 1.0)
            epsb = pers.tile([128, 1], F32, name="epsb", tag="epsb")
            nc.vector.memset(epsb[:], 1e-24)

            def evac(eng, out_ap, in_ap, bias=None, scale=None, relu=False):
                """PSUM -> SBUF evacuation on ACT or DVE."""
                if eng is nc.scalar:
                    func = AF.Relu if relu else (AF.Identity if bias is not None
                                                 else AF.Copy)
                    kw = {}
                    if bias is not None:
                        kw["bias"] = bias
                    if scale is not None:
                        kw["scale"] = scale
                    nc.scalar.activation(out_ap, in_ap, func, **kw)
                else:
                    if relu:
                        if scale is not None:
                            eng.tensor_scalar(out=out_ap, in0=in_ap,
                                              scalar1=0.0, scalar2=scale,
                                              op0=ALU.max, op1=ALU.mult)
                        else:
                            eng.tensor_scalar(out=out_ap, in0=in_ap,
                                              scalar1=0.0, scalar2=None,
                                              op0=ALU.max)
                    elif bias is not None:
                        eng.tensor_scalar_add(out_ap, in_ap, bias)
                    elif scale is not None:
                        eng.tensor_scalar_mul(out_ap, in_ap, scale)
                    else:
                        eng.tensor_copy(out_ap, in_ap)

            xts = {}

            def fetch(ch, nsplit=1):
                if ch >= CH or ch in xts:
                    return
                xt = xtp.tile([128, 2, 1024], F8, name="xt", tag="xt")
                w = 1024 // nsplit
                # chunk 0 spreads its minis over four DMA queues so the
                # issues are concurrent instead of serialized on SP
                qs = [nc.sync, nc.sync, nc.sync, nc.sync]
                for i in range(nsplit):
                    sl = slice(i * w, (i + 1) * w)
                    gl = slice(ch * 1024 + i * w, ch * 1024 + (i + 1) * w)
                    eng = qs[i % len(qs)] if nsplit > 1 else nc.sync
                    eng.dma_start(xt[:, :, sl], xTp_d[:, :, gl])
                xts[ch] = xt

            def emit_chunk(ch, wp, wpb, kv=None):
                """Stream one 1024-col slab of xTp; produce kTp, vp, cs."""
                kv = kv or kv_eng
                xt = xts.pop(ch)
                fetch(ch + 1)
                crd = wpb.tile([128, 8], F32, name="crd", tag="crd",
                                bufs=2)
                for jh in range(2):
                    ke = ksq_eng()
                    ksq = ksqp.tile([128, 2, 512], F8, name="ksq", tag="ksq")
                    kpss = []
                    for db in range(2):
                        kps = wp.tile([128, 512], F32, name="kps", tag="w")
                        kpss.append(kps)
                        for m in range(2):
                            lsl = slice(jh * 512 + m * 256,
                                        jh * 512 + (m + 1) * 256)
                            nc.tensor.matmul(kps[:, m * 256:(m + 1) * 256],
                                             wkt[:, :, db * 128:(db + 1) * 128],
                                             xt[:, :, lsl],
                                             start=True, stop=True,
                                             perf_mode=PM.DoubleRow)
                        gsl = slice(ch * 1024 + jh * 512,
                                    ch * 1024 + (jh + 1) * 512)
                        evac(kv(), kTp[:, db, gsl], kps[:],
                             bias=bk2[:, db:db + 1])
                        if ke == "S":
                            # squares straight off the psum on ACT: shorter
                            # chain (no wait on the k evacuation)
                            nc.scalar.activation(ksq[:, db, :], kps[:],
                                                 AF.Square,
                                                 bias=bk2[:, db:db + 1])
                    # ksq from the fp8 kT slab (SBUF), column norms via PE
                    gsl = slice(ch * 1024 + jh * 512, ch * 1024 + (jh + 1) * 512)
                    if ke != "S":
                        ke.tensor_tensor(out=ksq[:], in0=kTp[:, :, gsl],
                                         in1=kTp[:, :, gsl], op=ALU.mult)
                    for t in range(4):
                        col = jh * 4 + t
                        nc.tensor.matmul(crd[:, col:col + 1],
                                         ksq[:, :, t * 128:(t + 1) * 128],
                                         ones8[:],
                                         start=True, stop=True,
                                         perf_mode=PM.DoubleRow)
                # one cs batch per chunk (amortizes the fixed op costs)
                hs = slice(ch * 8, (ch + 1) * 8)
                csn = ep.tile([128, 8], F32, name="csn", tag="csn")
                nc.scalar.activation(csn[:], crd[:], AF.Sqrt, bias=epsb[:])
                nc.vector.reciprocal(cs[:, hs], csn[:])
                nc.gpsimd.tensor_copy(cs8[:, hs, 0:1], cs[:, hs])
                # v projection: [j, d] psums, 2 j-blocks per bank;
                # pure evac, then per-jb cs scaling in SBUF (Pool-friendly)
                for pj in range(4):
                    jj = ch * 4 + pj
                    vps = wp.tile([128, 512], F32, name="vps", tag="w")
                    for i in range(2):
                        lsl = slice((pj * 2 + i) * 128, (pj * 2 + i + 1) * 128)
                        nc.tensor.matmul(vps[:, i * 256:(i + 1) * 256],
                                         xt[:, :, lsl], wvt[:],
                                         start=True, stop=True,
                                         perf_mode=PM.DoubleRow)
                    evac(kv(), vp[jj][:], vps[:])
                    for i in range(2):
                        jb = 2 * jj + i
                        se = vs_eng()
                        if se is nc.scalar:
                            nc.scalar.activation(vp[jj][:, i, :],
                                                 vp[jj][:, i, :], AF.Copy,
                                                 scale=cs[:, jb:jb + 1])
                        else:
                            se.tensor_scalar(out=vp[jj][:, i, :],
                                             in0=vp[jj][:, i, :],
                                             scalar1=cs[:, jb:jb + 1],
                                             scalar2=None, op0=ALU.mult)

            def emit_rproj(wp):
                """q / k_self / v_self projections for this core's rows."""
                for db in range(2):
                    for rh in range(r // 512):
                        rsl = slice(rh * 512, (rh + 1) * 512)
                        qps = wp.tile([128, 512], F32, name="qps", tag="w")
                        for m in range(2):
                            msl = slice(rh * 512 + m * 256,
                                        rh * 512 + (m + 1) * 256)
                            nc.tensor.matmul(qps[:, m * 256:(m + 1) * 256],
                                             wqt[:, :, db * 128:(db + 1) * 128],
                                             xrTp[:, :, msl],
                                             start=True, stop=True,
                                             perf_mode=PM.DoubleRow)
                        evac(kv_eng(), qTp[:, db, rsl], qps[:],
                             bias=bq2[:, db:db + 1])
                        kps = wp.tile([128, 512], F32, name="ksps", tag="w")
                        for m in range(2):
                            msl = slice(rh * 512 + m * 256,
                                        rh * 512 + (m + 1) * 256)
                            nc.tensor.matmul(kps[:, m * 256:(m + 1) * 256],
                                             wkt[:, :, db * 128:(db + 1) * 128],
                                             xrTp[:, :, msl],
                                             start=True, stop=True,
                                             perf_mode=PM.DoubleRow)
                        evac(kv_eng(), ksf[:, db, rsl], kps[:],
                             bias=bk2[:, db:db + 1])
                for tt in range(RT // 2):
                    vsp = wp.tile([128, 512], F32, name="vsp", tag="w")
                    for i in range(2):
                        t = 2 * tt + i
                        nc.tensor.matmul(vsp[:, i * 256:(i + 1) * 256],
                                         xrTp[:, :, t * 128:(t + 1) * 128],
                                         wvt[:],
                                         start=True, stop=True,
                                         perf_mode=PM.DoubleRow)
                    evac(kv_eng(), vself[tt][:], vsp[:])

            def emit_selfterm(wpb):
                """m = relu(diag(q.k_self)) / |k_self| for diagonal removal."""
                for h in range(2):
                    hsl = slice(h * 512, (h + 1) * 512)
                    nc.gpsimd.tensor_tensor(out=qk8[:, :, hsl],
                                            in0=qTp[:, :, hsl],
                                            in1=ksf[:, :, hsl], op=ALU.mult)
                    if QS_PAT[h % len(QS_PAT)] == "A":
                        nc.scalar.activation(qs8[:, :, hsl], ksf[:, :, hsl],
                                             AF.Square)
                    else:
                        nc.gpsimd.tensor_tensor(out=qs8[:, :, hsl],
                                                in0=ksf[:, :, hsl],
                                                in1=ksf[:, :, hsl],
                                                op=ALU.mult)
                sdkp = wpb.tile([128, 2, RT], F32, name="sdkp", tag="crd",
                                bufs=2)
                for t in range(RT):
                    tsl = slice(t * 128, (t + 1) * 128)
                    nc.tensor.matmul(sdkp[:, 0, t:t + 1], qk8[:, :, tsl],
                                     ones8[:], start=True, stop=True,
                                     perf_mode=PM.DoubleRow)
                    nc.tensor.matmul(sdkp[:, 1, t:t + 1], qs8[:, :, tsl],
                                     ones8[:], start=True, stop=True,
                                     perf_mode=PM.DoubleRow)
                kst = ep.tile([128, RT], F32, name="kst", tag="kst")
                nc.scalar.activation(kst[:], sdkp[:, 1, :], AF.Sqrt,
                                     bias=epsb[:])
                inv = ep.tile([128, RT], F32, name="inv", tag="inv")
                nc.vector.reciprocal(inv[:], kst[:])
                nc.vector.tensor_scalar(out=msb[:], in0=sdkp[:, 0, :],
                                        scalar1=0.0, scalar2=None, op0=ALU.max)
                nc.gpsimd.tensor_tensor(out=msb[:], in0=msb[:], in1=inv[:],
                                        op=ALU.mult)

            def emit_scores_front(rb, g, sp, tag="sc", split_evac=False):
                """4 j-blocks of scores -> relu -> fp8 wt4."""
                rsl = slice(rb * RW, (rb + 1) * RW)
                sc = sp.tile([128, 1024], F32, name="sc", tag=tag)
                for i in range(GJB):
                    jb = g * GJB + i
                    nc.tensor.matmul(sc[:, i * 256:(i + 1) * 256],
                                     kTp[:, :, jb * 128:(jb + 1) * 128],
                                     qTp[:, :, rsl],
                                     start=True, stop=True,
                                     perf_mode=PM.DoubleRow)
                wt4 = wtp.tile([128, GJB, 256], F8, name="wt4", tag="wt4")
                if split_evac:
                    # halve the tail latency: both engines evacuate in parallel
                    evac(nc.scalar, wt4[:, 0:2, :], sc[:, 0:512], relu=True)
                    evac(nc.vector, wt4[:, 2:4, :], sc[:, 512:1024], relu=True)
                else:
                    evac(sc_eng(), wt4[:], sc[:], relu=True)
                return wt4

            def emit_av(g, wt4, avm, avo):
                for pj in range(GJB // 2):
                    jj = g * 2 + pj
                    for s in range(SS):
                        ssl = slice(s * 128, (s + 1) * 128)
                        nc.tensor.matmul(avm[:, s, :],
                                         wt4[:, 2 * pj:2 * pj + 2, ssl],
                                         vp[jj][:],
                                         start=(jj == 0),
                                         stop=(jj == NJ // 2 - 1),
                                         perf_mode=PM.DoubleRow)
                        nc.tensor.matmul(avo[:, s:s + 1],
                                         wt4[:, 2 * pj:2 * pj + 2, ssl],
                                         cs8[:, 2 * jj:2 * jj + 2, :],
                                         start=(jj == 0),
                                         stop=(jj == NJ // 2 - 1),
                                         perf_mode=PM.DoubleRow)

            def emit_scores(rb, g, sp, avm, avo, split_evac=False):
                wt4 = emit_scores_front(rb, g, sp, split_evac=split_evac)
                emit_av(g, wt4, avm, avo)

            def emit_epilogue(rb, avm, avo, split=False):
                avv = ep.tile([128, SS, 256], F32, name="avv", tag="avv")
                last = rb == NRB - 1
                if last:
                    # s=0 evacuates on ACT; s=1 skips the copy entirely (its
                    # numerator reads the accumulator PSUM directly below)
                    nc.scalar.activation(avv[:, 0, :], avm[:, 0, :], AF.Copy)
                else:
                    nc.scalar.activation(avv[:], avm[:], AF.Copy)
                for s in range(SS):
                    t = rb * SS + s
                    tmp = ep.tile([128, D], F32, name="tmp", tag="tmp")
                    nc.gpsimd.tensor_scalar(out=tmp[:],
                                            in0=vself[t // 2][:, t % 2, :],
                                            scalar1=msb[:, t:t + 1],
                                            scalar2=None, op0=ALU.mult)
                    den = ep.tile([128, 1], F32, name="den", tag="den")
                    nc.vector.tensor_scalar(out=den[:], in0=avo[:, s:s + 1],
                                            scalar1=msb[:, t:t + 1],
                                            scalar2=1e-9,
                                            op0=ALU.subtract, op1=ALU.add)
                    rec = ep.tile([128, 1], F32, name="rec", tag="rec")
                    nc.vector.reciprocal(rec[:], den[:])
                    # halved chains let the first output DMA fire early on
                    # the run's very last tile
                    ee = nc.gpsimd
                    ot = otp.tile([128, D], F32, name="ot", tag="ot")
                    num2 = ep.tile([128, D], F32, name="num2", tag="num2")
                    if last and s == SS - 1:
                        nc.vector.tensor_tensor(out=num2[:], in0=avm[:, s, :],
                                                in1=tmp[:], op=ALU.subtract)
                    else:
                        ee.tensor_tensor(out=num2[:], in0=avv[:, s, :],
                                         in1=tmp[:], op=ALU.subtract)
                    ot1 = ep.tile([128, D], F32, name="ot1", tag="ot1")
                    ee.tensor_scalar(out=ot1[:], in0=num2[:],
                                     scalar1=rec[:], scalar2=None,
                                     op0=ALU.mult)
                    ee.tensor_tensor(out=ot[:], in0=ot1[:],
                                     in1=xrt[t][:], op=ALU.add)
                    nc.sync.dma_start(out_d[t * 128:(t + 1) * 128, :], ot[:])

            # ---- phase A: all projections (PSUM: 2 k/q banks + 2x2 v
            # banks + 2 colsum banks) ----
            early = []
            NE = int(os.environ.get("K_NE", 0))
            with tc.tile_pool(name="wp", bufs=6, space="PSUM") as wp:
                with tc.tile_pool(name="wpb", bufs=2, space="PSUM") as wpb:
                    fetch(0, nsplit=int(os.environ.get("K_NS", 2)))
                    emit_chunk(0, wp, wpb)
                    emit_rproj(wp)
                    emit_selfterm(wpb)
                    for t in range(RT):
                        nc.gpsimd.dma_start(xrt[t][:],
                                            xr_d[t * 128:(t + 1) * 128, :])
                    nlast = int(os.environ.get("K_NL", 2))
                    for ch in range(1, CH - nlast):
                        emit_chunk(ch, wp, wpb)
                    for ch in range(CH - nlast, CH):
                        emit_chunk(ch, wp, wpb,
                                   kv=mk_cycle(os.environ.get("K_KVL", "ADA"),
                                               emap))
                if NE:
                    # the colsum banks free before the last v evacuations:
                    # run the first score group(s) there to overlap the drain
                    with tc.tile_pool(name="spe", bufs=1, space="PSUM") as spe:
                        for g in range(NE):
                            early.append(emit_scores_front(0, g, spe,
                                                           tag="scE"))

            # ---- phase B: scores + w@v (3 x 2-bank scores + 2 accum) ----
            with tc.tile_pool(name="sp", bufs=3, space="PSUM") as sp, \
                 tc.tile_pool(name="avp", bufs=1, space="PSUM") as avp:
                for rb in range(NRB):
                    avm = avp.tile([128, SS, 256], F32, name="avm", tag="avm")
                    avo = avp.tile([128, SS], F32, name="avo", tag="avo")
                    g0 = 0
                    if rb == 0:
                        for g, wt4 in enumerate(early):
                            emit_av(g, wt4, avm, avo)
                        g0 = len(early)
                    for g in range(g0, NG):
                        emit_scores(rb, g, sp, avm, avo,
                                    split_evac=(rb == NRB - 1 and
                                                g >= NG - int(os.environ.get(
                                                    "K_SE", 1))))
                    emit_epilogue(rb, avm, avo)
    nc.compile()
    return nc


def _get_nc(n=N, r=N // M):
    key = (n, r)
    if key not in _CACHE:
        _CACHE[key] = build(n, r)
    return _CACHE[key]


def _pairT(a2d):
    """[n, 256] -> fp8 pair layout [128, 2, n] (transposed)."""
    f8 = mybir.dt.np(F8)
    a = np.asarray(a2d, np.float32).astype(f8)
    n = a.shape[0]
    return np.ascontiguousarray(a.T.reshape(2, 128, n).transpose(1, 0, 2))


def kernel(x, Wq, bq, Wk, bk, Wv, bv):
    global LAST
    x = np.ascontiguousarray(np.asarray(x, np.float32))
    n = x.shape[0]
    r = n // M
    xTp = _pairT(x)
    wqTp = _pairT(np.asarray(Wq, np.float32))   # == Wq.T in pair layout
    wkTp = _pairT(np.asarray(Wk, np.float32))
    wvTp = _pairT(np.asarray(Wv, np.float32))
    xplus = x + np.asarray(bv, np.float32)[None, :]
    bq2 = np.ascontiguousarray(np.asarray(bq, np.float32).reshape(2, 128).T)
    bk2 = np.ascontiguousarray(np.asarray(bk, np.float32).reshape(2, 128).T)
    in_maps = []
    for c in range(M):
        rows = slice(c * r, (c + 1) * r)
        in_maps.append({
            "xTp": xTp,
            "xrTp": _pairT(x[rows]),
            "xr": np.ascontiguousarray(xplus[rows]),
            "wqTp": wqTp, "wkTp": wkTp, "wvTp": wvTp,
            "bq2": bq2, "bk2": bk2,
        })
    res = run_bass_kernel_spmd(_get_nc(n, r), in_maps, core_ids=list(range(M)),
                               trace=TRACE)
    LAST = res
    return np.concatenate([res.results[c]["out"] for c in range(M)], axis=0)
